# revision 17
# baseline (speedup 1.0000x reference)
"""Trainium2 Bass kernel for the note/wiki 3-way contraction + gate MLP.

Math (per note n):
    e[n]    = (wikivec * notevec[n]) @ W_emb.T + b_emb          # (C, K)
    attn[n] = sigmoid(e[n] @ W_att.T + b_att)                   # (C, K)
    s[n]    = sum_k attn[n]*e[n]*W_out[0,k] + b_out             # (C,)

Sharding: data-parallel over the 16 notes -> 2 notes per core on 8 cores.
wikivec / W_emb are replicated (pre-transposed, zero-padded to 10112 = 79*128
along the contraction axis, cast to bf16 on the host; ~10 MB per core streams
HBM->SBUF underneath the PE work and stays resident -- no buffer rotation).

Device schedule (v-major so the contraction dim sits on partitions):
  - consts ride the idle GpSimd SWDGE queue; all wiki/wemb block DMAs are
    issued up front on the Sync HWDGE queue (big blocks, few issues).
  - the PE is pre-warmed with zero-tile matmuls that accumulate 0 into the
    real e^T PSUM banks, so HAM is at full clock when real data lands.
  - per v-tile, ONE engine scales wikivec^T by both notes' scalars
    (DVE 3 tiles : ACT 1 tile) -> single-producer mov tiles, fewer waits.
  - 2 matmuls per v-tile accumulate e^T[k, (note,c)] into two PSUM banks.
  - phase 2 (bias, attn logits, sigmoid, gate, W_out) runs in bf16 with the
    sigmoid table preloaded at kernel start and dummy matmuls plugging the
    PE-idle windows so the wout matmuls run warm.
"""

import sys

if "/opt/trn_rl_repo" not in sys.path:
    sys.path.insert(0, "/opt/trn_rl_repo")

import numpy as np
import ml_dtypes

import concourse.bass as bass
import concourse.mybir as mybir
import concourse.tile as tile
from concourse import bacc
from concourse.bass_utils import run_bass_kernel_spmd

N_CORES = 8
N, C, V, K = 16, 256, 10000, 256
J = 79  # number of 128-row v-tiles (V padded to 10112)
J2 = 80  # scales stride (multiple of 16)
NLOC = N // N_CORES  # notes per core
NC2 = NLOC * C  # 512: (note, c) column block
BLOCKS = [4] * 19 + [3]  # v-tiles per DMA block, sum = 79
CK = C + K  # packed block stride: wiki cols then wemb cols per v-tile
WARM = 5  # zero v-tiles of PE warmup (2 matmuls each)

F32 = mybir.dt.float32
BF16 = mybir.dt.bfloat16
BF16_NP = ml_dtypes.bfloat16

# fp32 const pack column offsets
SC_OFF = 0  # scales [128, NLOC*J2]
WOUT_OFF = SC_OFF + NLOC * J2  # [128, 2]  (bf16 value, stored f32)
BE_OFF = WOUT_OFF + 2  # [128, 2]
BA_OFF = BE_OFF + 2  # [128, 2]
BO_OFF = BA_OFF + 2  # [128, 1] (b_out replicated)
CST_COLS = BO_OFF + 1

_NC_CACHE = {}


def _build_nc():
    nc = bacc.Bacc(None, target_bir_lowering=False)

    wwT = nc.declare_dram_parameter("wwT", [128, J * CK], BF16, isOutput=False)
    cstF = nc.declare_dram_parameter("cstF", [128, CST_COLS], F32, isOutput=False)
    watT = nc.declare_dram_parameter("watT", [128, 2 * K], BF16, isOutput=False)
    woutB = nc.declare_dram_parameter("woutB", [128, 2], BF16, isOutput=False)
    s_out = nc.declare_dram_parameter("s_out", [1, NLOC * C], F32, isOutput=True)

    with tile.TileContext(nc) as tc:
        with (
            tc.tile_pool(name="const", bufs=1) as constp,
            tc.tile_pool(name="mov", bufs=8) as movp,
            tc.tile_pool(name="post", bufs=1) as postp,
            tc.tile_pool(name="psum", bufs=1, space="PSUM") as psp,
        ):
            # zero tiles for PE warmup / phase-2 filler matmuls -- memset on
            # DVE so they're ready immediately (GpSimd stays fully idle)
            zet = constp.tile([128, K], BF16)
            nc.vector.memset(zet[:], 0)
            zmov = constp.tile([128, NC2], BF16)
            nc.vector.memset(zmov[:], 0)

            # preload the sigmoid table set once, at kernel start
            sigw = constp.tile([128, 1], F32)
            nc.scalar.activation(
                sigw[:], zet[:, 0:1], mybir.ActivationFunctionType.Sigmoid
            )

            # ---- all DMAs up front on Sync, ordered for fastest start ----
            # wiki+wemb are packed per v-tile in one DRAM param so each
            # block needs a single dma_start (per-tile cols: C wiki, K wemb)
            cst = constp.tile([128, CST_COLS], F32)
            nc.sync.dma_start(cst[:], cstF[:])
            ww_b = []
            offs = []
            off = 0
            for b, nb in enumerate(BLOCKS):
                ww = constp.tile(
                    [128, nb * CK], BF16, name=f"ww{off}", tag=f"ww{off}"
                )
                nc.sync.dma_start(ww[:], wwT[:, off * CK : (off + nb) * CK])
                ww_b.append(ww)
                offs.append(off)
                off += nb
            wat = constp.tile([128, 2 * K], BF16)
            nc.sync.dma_start(wat[:], watT[:])
            wout = constp.tile([128, 2], BF16)
            nc.sync.dma_start(wout[:], woutB[:])

            # e^T accumulators: [k-half 128, (note,c) 512] fp32, one bank each
            e_ps = [
                psp.tile([128, NC2], F32, name=f"e_ps{m}", tag=f"e_ps{m}")
                for m in range(2)
            ]

            # PE warmup: accumulate 0 into e_ps so HAM is warm for real work
            for w in range(WARM):
                for m in range(2):
                    nc.tensor.matmul(
                        e_ps[m][:],
                        zet[:, m * 128 : (m + 1) * 128],
                        zmov[:],
                        start=(w == 0),
                        stop=False,
                    )

            sc = cst[:, SC_OFF : SC_OFF + NLOC * J2]

            # ---- phase 1: scale + accumulate over all 79 v-tiles ----
            # DVE scales 3 of every 5 v-tiles, ACT the other 2 (both notes
            # on one engine -> single-producer mov tiles).  The last TAIL
            # v-tiles emit all m=0 matmuls before the m=1 ones so the m=0
            # bias/cast of phase 2 overlaps the final m=1 accumulation.
            TAIL = 3
            tail_mm = []
            jg = 0
            for b, nb in enumerate(BLOCKS):
                for jj in range(nb):
                    j = jg + jj
                    wts = ww_b[b][:, jj * CK : jj * CK + C]
                    mov = movp.tile([128, NC2], BF16)
                    if j % 5 < 3:
                        nc.vector.tensor_scalar_mul(
                            mov[:, 0:C], wts, sc[:, j : j + 1]
                        )
                        nc.vector.tensor_scalar_mul(
                            mov[:, C : 2 * C], wts, sc[:, J2 + j : J2 + j + 1]
                        )
                    else:
                        nc.scalar.mul(mov[:, 0:C], wts, mul=sc[:, j : j + 1])
                        nc.scalar.mul(
                            mov[:, C : 2 * C], wts, mul=sc[:, J2 + j : J2 + j + 1]
                        )
                    sp = j == J - 1
                    lhs = lambda m, _b=b, _jj=jj: ww_b[_b][
                        :,
                        _jj * CK + C + m * 128 : _jj * CK + C + (m + 1) * 128,
                    ]
                    nc.tensor.matmul(
                        e_ps[0][:], lhs(0), mov[:], start=False, stop=sp
                    )
                    if j >= J - TAIL:
                        tail_mm.append((lhs(1), mov, sp))
                    else:
                        nc.tensor.matmul(
                            e_ps[1][:], lhs(1), mov[:], start=False, stop=False
                        )
                jg += nb
            for lhs1, mov, sp in tail_mm:
                nc.tensor.matmul(e_ps[1][:], lhs1, mov[:], start=False, stop=sp)

            # ---- phase 2: bias, attn logits, sigmoid, gate, W_out ----
            be = cst[:, BE_OFF : BE_OFF + 2]
            ba = cst[:, BA_OFF : BA_OFF + 2]
            bo = cst[:, BO_OFF : BO_OFF + 1]

            zdum = psp.tile([128, NC2], F32, name="zdum", tag="zdum")

            def filler(n):
                # zero matmuls that keep the PE clock warm through phase-2
                # dependency stalls (zdum is read at the end so these are
                # not dead-code-eliminated)
                for _ in range(n):
                    nc.tensor.matmul(
                        zdum[:], zet[:, 0:128], zmov[:], start=True, stop=True
                    )

            # PE busy while ACT/DVE run the m0 bias + bf16 cast
            filler(2)

            eb = []
            for m in range(2):
                ef_m = postp.tile([128, NC2], F32, tag=f"ef{m}")
                nc.scalar.activation(
                    ef_m[:],
                    e_ps[m][:],
                    mybir.ActivationFunctionType.Identity,
                    bias=be[:, m : m + 1],
                    scale=1.0,
                )
                eb_m = postp.tile([128, NC2], BF16, tag=f"eb{m}")
                nc.vector.tensor_copy(eb_m[:], ef_m[:])
                eb.append(eb_m)

            a_ps = [
                psp.tile([128, NC2], F32, name=f"a_ps{jm}", tag=f"a_ps{jm}")
                for jm in range(2)
            ]
            for kt in range(2):
                for jm in range(2):
                    nc.tensor.matmul(
                        a_ps[jm][:],
                        wat[:, kt * K + jm * 128 : kt * K + (jm + 1) * 128],
                        eb[kt][:],
                        start=(kt == 0),
                        stop=(kt == 1),
                    )

            s_ps = psp.tile([1, NC2], F32, tag="s_ps")
            for jm in range(2):
                atn = postp.tile([128, NC2], BF16, tag=f"atn{jm}")
                nc.scalar.activation(
                    atn[:],
                    a_ps[jm][:],
                    mybir.ActivationFunctionType.Sigmoid,
                    bias=ba[:, jm : jm + 1],
                    scale=1.0,
                )
                v_jm = postp.tile([128, NC2], BF16, tag=f"v{jm}")
                nc.vector.tensor_mul(v_jm[:], atn[:], eb[jm][:])
                if jm == 0:
                    # PE stays warm while ACT runs sigmoid 0 + DVE gates
                    filler(3)
                nc.tensor.matmul(
                    s_ps[:],
                    wout[:, jm : jm + 1],
                    v_jm[:],
                    start=(jm == 0),
                    stop=(jm == 1),
                )
                if jm == 0:
                    filler(1)
            s_sb = postp.tile([1, NC2], F32, tag="s_sb")
            nc.scalar.add(s_sb[:], s_ps[:], bo[0:1, 0:1])
            nc.sync.dma_start(s_out[:], s_sb[:])
            # liveness anchor for the filler matmuls
            zrd = postp.tile([1, 1], F32, tag="zrd")
            nc.vector.tensor_copy(zrd[:], zdum[0:1, 0:1])

    nc.compile()
    return nc


def _get_nc():
    if "nc" not in _NC_CACHE:
        _NC_CACHE["nc"] = _build_nc()
    return _NC_CACHE["nc"]


def _pack_ww(wiki, wemb):
    """-> [128, J*CK] bf16; per v-tile j: C wiki cols then K wemb cols,
    col value at partition p is a[c, j*128+p] (zero-padded past V)."""
    out = np.zeros((J * 128, CK), np.float32)
    out[:V, :C] = np.asarray(wiki, np.float32).T
    out[:V, C:] = np.asarray(wemb, np.float32).T
    out = out.reshape(J, 128, CK).transpose(1, 0, 2)
    return np.ascontiguousarray(out.reshape(128, J * CK)).astype(BF16_NP)


def prep_inputs(notevec, wikivec, W_emb, b_emb, W_att, b_att, W_out, b_out):
    wwT = _pack_ww(wikivec, W_emb)
    # watT[p, kt*K + jm*128 + q] must hold W_att[jm*128+q, kt*128+p]
    wa = np.asarray(W_att, np.float32)  # (j, k)
    watT = np.zeros((128, 2 * K), np.float32)
    for kt in range(2):
        for jm in range(2):
            watT[:, kt * K + jm * 128 : kt * K + (jm + 1) * 128] = wa[
                jm * 128 : (jm + 1) * 128, kt * 128 : (kt + 1) * 128
            ].T
    watT = watT.astype(BF16_NP)
    woutB = (
        np.ascontiguousarray(np.asarray(W_out, np.float32)[0].reshape(2, 128).T)
        .astype(BF16_NP)
    )

    nv = np.zeros((N, J2 * 128), np.float32)
    nv[:, :V] = np.asarray(notevec, np.float32)
    bemb = np.asarray(b_emb, np.float32).reshape(2, 128).T
    batt = np.asarray(b_att, np.float32).reshape(2, 128).T
    bo = float(np.asarray(b_out, np.float32).reshape(1)[0])

    in_maps = []
    for i in range(N_CORES):
        cst = np.zeros((128, CST_COLS), np.float32)
        # scales[p, l*J2 + j] = notevec[NLOC*i+l, j*128+p]
        scl = nv[i * NLOC : (i + 1) * NLOC].reshape(NLOC, J2, 128).transpose(2, 0, 1)
        cst[:, SC_OFF : SC_OFF + NLOC * J2] = scl.reshape(128, NLOC * J2)
        cst[:, BE_OFF : BE_OFF + 2] = bemb
        cst[:, BA_OFF : BA_OFF + 2] = batt
        cst[:, BO_OFF] = bo
        in_maps.append(
            {
                "wwT": wwT,
                "cstF": np.ascontiguousarray(cst),
                "watT": watT,
                "woutB": woutB,
            }
        )
    return in_maps


def run(in_maps, **kw):
    nc = _get_nc()
    return run_bass_kernel_spmd(nc, in_maps, list(range(N_CORES)), **kw)


def kernel(notevec, wikivec, W_emb, b_emb, W_att, b_att, W_out, b_out):
    in_maps = prep_inputs(
        notevec, wikivec, W_emb, b_emb, W_att, b_att, W_out, b_out
    )
    res = run(in_maps)
    out = np.concatenate(
        [r["s_out"].reshape(NLOC, C) for r in res.results], axis=0
    )
    return out.astype(np.float32)


# revision 20
# speedup vs baseline: 1.0357x; 1.0357x over previous
"""Trainium2 Bass kernel for the note/wiki 3-way contraction + gate MLP.

Math (per note n):
    e[n]    = (wikivec * notevec[n]) @ W_emb.T + b_emb          # (C, K)
    attn[n] = sigmoid(e[n] @ W_att.T + b_att)                   # (C, K)
    s[n]    = sum_k attn[n]*e[n]*W_out[0,k] + b_out             # (C,)

Sharding: data-parallel over the 16 notes -> 2 notes per core on 8 cores.
wikivec / W_emb are replicated (pre-transposed, zero-padded to 10112 = 79*128
along the contraction axis, cast to bf16 on the host; ~10 MB per core streams
HBM->SBUF underneath the PE work and stays resident -- no buffer rotation).

Device schedule (v-major so the contraction dim sits on partitions):
  - consts ride the idle GpSimd SWDGE queue; all wiki/wemb block DMAs are
    issued up front on the Sync HWDGE queue (big blocks, few issues).
  - the PE is pre-warmed with zero-tile matmuls that accumulate 0 into the
    real e^T PSUM banks, so HAM is at full clock when real data lands.
  - per v-tile, ONE engine scales wikivec^T by both notes' scalars
    (DVE 3 tiles : ACT 1 tile) -> single-producer mov tiles, fewer waits.
  - 2 matmuls per v-tile accumulate e^T[k, (note,c)] into two PSUM banks.
  - phase 2 (bias, attn logits, sigmoid, gate, W_out) runs in bf16 with the
    sigmoid table preloaded at kernel start and dummy matmuls plugging the
    PE-idle windows so the wout matmuls run warm.
"""

import sys

if "/opt/trn_rl_repo" not in sys.path:
    sys.path.insert(0, "/opt/trn_rl_repo")

import numpy as np
import ml_dtypes

import concourse.bass as bass
import concourse.mybir as mybir
import concourse.tile as tile
from concourse import bacc
from concourse.bass_utils import run_bass_kernel_spmd

N_CORES = 8
N, C, V, K = 16, 256, 10000, 256
J = 79  # number of 128-row v-tiles (V padded to 10112)
J2 = 80  # scales stride (multiple of 16)
NLOC = N // N_CORES  # notes per core
NC2 = NLOC * C  # 512: (note, c) column block
BLOCKS = [5] + [8] * 9 + [2]  # v-tiles per DMA block, sum = 79
CK = C + K  # packed block stride: wiki cols then wemb cols per v-tile
WARM = 5  # zero v-tiles of PE warmup (2 matmuls each)

F32 = mybir.dt.float32
BF16 = mybir.dt.bfloat16
BF16_NP = ml_dtypes.bfloat16

# fp32 const pack column offsets
SC_OFF = 0  # scales [128, NLOC*J2]
WOUT_OFF = SC_OFF + NLOC * J2  # [128, 2]  (bf16 value, stored f32)
BE_OFF = WOUT_OFF + 2  # [128, 2]
BA_OFF = BE_OFF + 2  # [128, 2]
BO_OFF = BA_OFF + 2  # [128, 1] (b_out replicated)
CST_COLS = BO_OFF + 1

_NC_CACHE = {}


def _build_nc():
    nc = bacc.Bacc(None, target_bir_lowering=False)

    wwT = nc.declare_dram_parameter("wwT", [128, J * CK], BF16, isOutput=False)
    cstF = nc.declare_dram_parameter("cstF", [128, CST_COLS], F32, isOutput=False)
    watT = nc.declare_dram_parameter("watT", [128, 2 * K], BF16, isOutput=False)
    woutB = nc.declare_dram_parameter("woutB", [128, 2], BF16, isOutput=False)
    s_out = nc.declare_dram_parameter("s_out", [1, NLOC * C], F32, isOutput=True)

    with tile.TileContext(nc) as tc:
        with (
            tc.tile_pool(name="const", bufs=1) as constp,
            tc.tile_pool(name="mov", bufs=8) as movp,
            tc.tile_pool(name="post", bufs=1) as postp,
            tc.tile_pool(name="psum", bufs=1, space="PSUM") as psp,
        ):
            # zero tiles for PE warmup / phase-2 filler matmuls -- memset on
            # DVE so they're ready immediately (GpSimd stays fully idle)
            zet = constp.tile([128, K], BF16)
            nc.vector.memset(zet[:], 0)
            zmov = constp.tile([128, NC2], BF16)
            nc.vector.memset(zmov[:], 0)

            # preload the sigmoid table set once, at kernel start
            sigw = constp.tile([128, 1], F32)
            nc.scalar.activation(
                sigw[:], zet[:, 0:1], mybir.ActivationFunctionType.Sigmoid
            )

            # ---- all DMAs up front on Sync, ordered for fastest start ----
            # wiki+wemb are packed per v-tile in one DRAM param so each
            # block needs a single dma_start (per-tile cols: C wiki, K wemb)
            cst = constp.tile([128, CST_COLS], F32)
            nc.sync.dma_start(cst[:], cstF[:])
            ww_b = []
            offs = []
            off = 0
            for b, nb in enumerate(BLOCKS):
                ww = constp.tile(
                    [128, nb * CK], BF16, name=f"ww{off}", tag=f"ww{off}"
                )
                nc.sync.dma_start(ww[:], wwT[:, off * CK : (off + nb) * CK])
                ww_b.append(ww)
                offs.append(off)
                off += nb
            wat = constp.tile([128, 2 * K], BF16)
            nc.sync.dma_start(wat[:], watT[:])
            wout = constp.tile([128, 2], BF16)
            nc.sync.dma_start(wout[:], woutB[:])

            # e^T accumulators: [k-half 128, (note,c) 512] fp32, one bank each
            e_ps = [
                psp.tile([128, NC2], F32, name=f"e_ps{m}", tag=f"e_ps{m}")
                for m in range(2)
            ]

            # PE warmup: accumulate 0 into e_ps so HAM is warm for real work
            for w in range(WARM):
                for m in range(2):
                    nc.tensor.matmul(
                        e_ps[m][:],
                        zet[:, m * 128 : (m + 1) * 128],
                        zmov[:],
                        start=(w == 0),
                        stop=False,
                    )

            sc = cst[:, SC_OFF : SC_OFF + NLOC * J2]

            # ---- phase 1: scale + accumulate over all 79 v-tiles ----
            # DVE scales 3 of every 5 v-tiles, ACT the other 2 (both notes
            # on one engine -> single-producer mov tiles).  The last TAIL
            # v-tiles emit all m=0 matmuls before the m=1 ones so the m=0
            # bias/cast of phase 2 overlaps the final m=1 accumulation.
            TAIL = 3
            tail_mm = []
            jg = 0
            for b, nb in enumerate(BLOCKS):
                for jj in range(nb):
                    j = jg + jj
                    wts = ww_b[b][:, jj * CK : jj * CK + C]
                    mov = movp.tile([128, NC2], BF16)
                    if j % 5 < 3:
                        nc.vector.tensor_scalar_mul(
                            mov[:, 0:C], wts, sc[:, j : j + 1]
                        )
                        nc.vector.tensor_scalar_mul(
                            mov[:, C : 2 * C], wts, sc[:, J2 + j : J2 + j + 1]
                        )
                    else:
                        nc.scalar.mul(mov[:, 0:C], wts, mul=sc[:, j : j + 1])
                        nc.scalar.mul(
                            mov[:, C : 2 * C], wts, mul=sc[:, J2 + j : J2 + j + 1]
                        )
                    sp = j == J - 1
                    lhs = lambda m, _b=b, _jj=jj: ww_b[_b][
                        :,
                        _jj * CK + C + m * 128 : _jj * CK + C + (m + 1) * 128,
                    ]
                    nc.tensor.matmul(
                        e_ps[0][:], lhs(0), mov[:], start=False, stop=sp
                    )
                    if j >= J - TAIL:
                        tail_mm.append((lhs(1), mov, sp))
                    else:
                        nc.tensor.matmul(
                            e_ps[1][:], lhs(1), mov[:], start=False, stop=False
                        )
                jg += nb
            for lhs1, mov, sp in tail_mm:
                nc.tensor.matmul(e_ps[1][:], lhs1, mov[:], start=False, stop=sp)

            # ---- phase 2: bias, attn logits, sigmoid, gate, W_out ----
            be = cst[:, BE_OFF : BE_OFF + 2]
            ba = cst[:, BA_OFF : BA_OFF + 2]
            bo = cst[:, BO_OFF : BO_OFF + 1]

            zdum = psp.tile([128, NC2], F32, name="zdum", tag="zdum")

            NFILL = 6
            fill_i = [0]

            def filler(n):
                # zero matmuls that keep the PE clock warm through phase-2
                # dependency stalls -- one accumulation chain into zdum that
                # is read at the end, so none is dead-code-eliminated
                for _ in range(n):
                    nc.tensor.matmul(
                        zdum[:],
                        zet[:, 0:128],
                        zmov[:],
                        start=(fill_i[0] == 0),
                        stop=(fill_i[0] == NFILL - 1),
                    )
                    fill_i[0] += 1

            # PE busy while DVE/ACT cast e^T halves to bf16 straight from
            # PSUM (b_emb's effect on the attn logits is folded into the
            # b_att column on the host, so no bias pass is needed here)
            filler(2)

            eb0 = postp.tile([128, NC2], BF16, tag="eb0")
            nc.vector.tensor_copy(eb0[:], e_ps[0][:])
            eb1 = postp.tile([128, NC2], BF16, tag="eb1")
            nc.scalar.copy(eb1[:], e_ps[1][:])
            eb = [eb0, eb1]

            a_ps = [
                psp.tile([128, NC2], F32, name=f"a_ps{jm}", tag=f"a_ps{jm}")
                for jm in range(2)
            ]
            for jm in range(2):
                for kt in range(2):
                    nc.tensor.matmul(
                        a_ps[jm][:],
                        wat[:, kt * K + jm * 128 : kt * K + (jm + 1) * 128],
                        eb[kt][:],
                        start=(kt == 0),
                        stop=(kt == 1),
                    )

            # gate input e + b_emb, cast bf16 (off the critical path)
            ef = []
            for m in range(2):
                ef_m = postp.tile([128, NC2], BF16, tag=f"ef{m}")
                nc.vector.tensor_scalar_add(ef_m[:], e_ps[m][:], be[:, m : m + 1])
                ef.append(ef_m)

            s_ps = psp.tile([1, NC2], F32, tag="s_ps")
            for jm in range(2):
                atn = postp.tile([128, NC2], BF16, tag=f"atn{jm}")
                nc.scalar.activation(
                    atn[:],
                    a_ps[jm][:],
                    mybir.ActivationFunctionType.Sigmoid,
                    bias=ba[:, jm : jm + 1],
                    scale=1.0,
                )
                v_jm = postp.tile([128, NC2], BF16, tag=f"v{jm}")
                nc.vector.tensor_mul(v_jm[:], atn[:], ef[jm][:])
                if jm == 0:
                    # PE stays warm while ACT runs sigmoid 0 + DVE gates
                    filler(3)
                nc.tensor.matmul(
                    s_ps[:],
                    wout[:, jm : jm + 1],
                    v_jm[:],
                    start=(jm == 0),
                    stop=(jm == 1),
                )
                if jm == 0:
                    filler(1)
            s_sb = postp.tile([1, NC2], F32, tag="s_sb")
            nc.scalar.add(s_sb[:], s_ps[:], bo[0:1, 0:1])
            nc.sync.dma_start(s_out[:], s_sb[:])
            # liveness anchor for the filler matmuls
            zrd = postp.tile([1, 1], F32, tag="zrd")
            nc.vector.tensor_copy(zrd[:], zdum[0:1, 0:1])

    nc.compile()
    return nc


def _get_nc():
    if "nc" not in _NC_CACHE:
        _NC_CACHE["nc"] = _build_nc()
    return _NC_CACHE["nc"]


def _pack_ww(wiki, wemb):
    """-> [128, J*CK] bf16; per v-tile j: C wiki cols then K wemb cols,
    col value at partition p is a[c, j*128+p] (zero-padded past V)."""
    out = np.zeros((J * 128, CK), np.float32)
    out[:V, :C] = np.asarray(wiki, np.float32).T
    out[:V, C:] = np.asarray(wemb, np.float32).T
    out = out.reshape(J, 128, CK).transpose(1, 0, 2)
    return np.ascontiguousarray(out.reshape(128, J * CK)).astype(BF16_NP)


def prep_inputs(notevec, wikivec, W_emb, b_emb, W_att, b_att, W_out, b_out):
    wwT = _pack_ww(wikivec, W_emb)
    # watT[p, kt*K + jm*128 + q] must hold W_att[jm*128+q, kt*128+p]
    wa = np.asarray(W_att, np.float32)  # (j, k)
    watT = np.zeros((128, 2 * K), np.float32)
    for kt in range(2):
        for jm in range(2):
            watT[:, kt * K + jm * 128 : kt * K + (jm + 1) * 128] = wa[
                jm * 128 : (jm + 1) * 128, kt * 128 : (kt + 1) * 128
            ].T
    watT = watT.astype(BF16_NP)
    woutB = (
        np.ascontiguousarray(np.asarray(W_out, np.float32)[0].reshape(2, 128).T)
        .astype(BF16_NP)
    )

    nv = np.zeros((N, J2 * 128), np.float32)
    nv[:, :V] = np.asarray(notevec, np.float32)
    bemb = np.asarray(b_emb, np.float32).reshape(2, 128).T
    # b_emb's contribution to the attn logits, folded into b_att
    # (uses the bf16-rounded W_att actually applied on device)
    ba2 = np.asarray(b_att, np.float32) + wa.astype(BF16_NP).astype(
        np.float32
    ) @ np.asarray(b_emb, np.float32)
    batt = ba2.reshape(2, 128).T
    bo = float(np.asarray(b_out, np.float32).reshape(1)[0])

    in_maps = []
    for i in range(N_CORES):
        cst = np.zeros((128, CST_COLS), np.float32)
        # scales[p, l*J2 + j] = notevec[NLOC*i+l, j*128+p]
        scl = nv[i * NLOC : (i + 1) * NLOC].reshape(NLOC, J2, 128).transpose(2, 0, 1)
        cst[:, SC_OFF : SC_OFF + NLOC * J2] = scl.reshape(128, NLOC * J2)
        cst[:, BE_OFF : BE_OFF + 2] = bemb
        cst[:, BA_OFF : BA_OFF + 2] = batt
        cst[:, BO_OFF] = bo
        in_maps.append(
            {
                "wwT": wwT,
                "cstF": np.ascontiguousarray(cst),
                "watT": watT,
                "woutB": woutB,
            }
        )
    return in_maps


def run(in_maps, **kw):
    nc = _get_nc()
    return run_bass_kernel_spmd(nc, in_maps, list(range(N_CORES)), **kw)


def kernel(notevec, wikivec, W_emb, b_emb, W_att, b_att, W_out, b_out):
    in_maps = prep_inputs(
        notevec, wikivec, W_emb, b_emb, W_att, b_att, W_out, b_out
    )
    res = run(in_maps)
    out = np.concatenate(
        [r["s_out"].reshape(NLOC, C) for r in res.results], axis=0
    )
    return out.astype(np.float32)


# revision 26
# speedup vs baseline: 1.0411x; 1.0052x over previous
"""Trainium2 Bass kernel for the note/wiki 3-way contraction + gate MLP.

Math (per note n):
    e[n]    = (wikivec * notevec[n]) @ W_emb.T + b_emb          # (C, K)
    attn[n] = sigmoid(e[n] @ W_att.T + b_att)                   # (C, K)
    s[n]    = sum_k attn[n]*e[n]*W_out[0,k] + b_out             # (C,)

Sharding: data-parallel over the 16 notes -> 2 notes per core on 8 cores.
wikivec / W_emb are replicated (pre-transposed, zero-padded to 10112 = 79*128
along the contraction axis, cast to bf16 on the host; ~10 MB per core streams
HBM->SBUF underneath the PE work and stays resident -- no buffer rotation).

Device schedule (v-major so the contraction dim sits on partitions):
  - consts ride the idle GpSimd SWDGE queue; all wiki/wemb block DMAs are
    issued up front on the Sync HWDGE queue (big blocks, few issues).
  - the PE is pre-warmed with zero-tile matmuls that accumulate 0 into the
    real e^T PSUM banks, so HAM is at full clock when real data lands.
  - per v-tile, ONE engine scales wikivec^T by both notes' scalars
    (DVE 3 tiles : ACT 1 tile) -> single-producer mov tiles, fewer waits.
  - 2 matmuls per v-tile accumulate e^T[k, (note,c)] into two PSUM banks.
  - phase 2 (bias, attn logits, sigmoid, gate, W_out) runs in bf16 with the
    sigmoid table preloaded at kernel start and dummy matmuls plugging the
    PE-idle windows so the wout matmuls run warm.
"""

import sys

if "/opt/trn_rl_repo" not in sys.path:
    sys.path.insert(0, "/opt/trn_rl_repo")

import numpy as np
import ml_dtypes

import concourse.bass as bass
import concourse.mybir as mybir
import concourse.tile as tile
from concourse import bacc
from concourse.bass_utils import run_bass_kernel_spmd

N_CORES = 8
N, C, V, K = 16, 256, 10000, 256
J = 79  # number of 128-row v-tiles (V padded to 10112)
J2 = 80  # scales stride (multiple of 16)
NLOC = N // N_CORES  # notes per core
NC2 = NLOC * C  # 512: (note, c) column block
BLOCKS = [5] + [8] * 9 + [2]  # v-tiles per DMA block, sum = 79
CK = C + K  # packed block stride: wiki cols then wemb cols per v-tile
WARM = 4  # zero v-tiles of PE warmup (2 matmuls each)

F32 = mybir.dt.float32
BF16 = mybir.dt.bfloat16
BF16_NP = ml_dtypes.bfloat16

# fp32 const pack column offsets
SC_OFF = 0  # scales [128, NLOC*J2]
WOUT_OFF = SC_OFF + NLOC * J2  # [128, 2]  (bf16 value, stored f32)
BE_OFF = WOUT_OFF + 2  # [128, 2]
BA_OFF = BE_OFF + 2  # [128, 2]
BO_OFF = BA_OFF + 2  # [128, 1] (b_out replicated)
CST_COLS = BO_OFF + 1

_NC_CACHE = {}


def _build_nc():
    nc = bacc.Bacc(None, target_bir_lowering=False)

    wwT = nc.declare_dram_parameter("wwT", [128, J * CK], BF16, isOutput=False)
    cstF = nc.declare_dram_parameter("cstF", [128, CST_COLS], F32, isOutput=False)
    watT = nc.declare_dram_parameter("watT", [128, 2 * K], BF16, isOutput=False)
    woutB = nc.declare_dram_parameter("woutB", [128, 2], BF16, isOutput=False)
    s_out = nc.declare_dram_parameter("s_out", [1, NLOC * C], F32, isOutput=True)

    with tile.TileContext(nc) as tc:
        with (
            tc.tile_pool(name="const", bufs=1) as constp,
            tc.tile_pool(name="mov", bufs=8) as movp,
            tc.tile_pool(name="post", bufs=1) as postp,
            tc.tile_pool(name="psum", bufs=1, space="PSUM") as psp,
        ):
            # zero tiles for PE warmup / phase-2 filler matmuls -- memset on
            # the otherwise idle GpSimd so the warmup starts ASAP
            zet = constp.tile([128, K], BF16)
            nc.gpsimd.memset(zet[:], 0)
            zmov = constp.tile([128, NC2], BF16)
            nc.gpsimd.memset(zmov[:], 0)

            # preload the sigmoid table set once, at kernel start
            sigw = constp.tile([128, 1], F32)
            nc.scalar.activation(
                sigw[:], zet[:, 0:1], mybir.ActivationFunctionType.Sigmoid
            )

            # ---- all DMAs up front on Sync, ordered for fastest start ----
            # wiki+wemb are packed per v-tile in one DRAM param so each
            # block needs a single dma_start (per-tile cols: C wiki, K wemb)
            cst = constp.tile([128, CST_COLS], F32)
            nc.sync.dma_start(cst[:], cstF[:])
            ww_b = []
            offs = []
            off = 0
            for b, nb in enumerate(BLOCKS):
                ww = constp.tile(
                    [128, nb * CK], BF16, name=f"ww{off}", tag=f"ww{off}"
                )
                nc.sync.dma_start(ww[:], wwT[:, off * CK : (off + nb) * CK])
                ww_b.append(ww)
                offs.append(off)
                off += nb
            wat = constp.tile([128, 2 * K], BF16)
            nc.sync.dma_start(wat[:], watT[:])
            wout = constp.tile([128, 2], BF16)
            nc.sync.dma_start(wout[:], woutB[:])

            # e^T accumulators: [k-half 128, (note,c) 512] fp32, one bank each
            e_ps = [
                psp.tile([128, NC2], F32, name=f"e_ps{m}", tag=f"e_ps{m}")
                for m in range(2)
            ]

            # PE warmup: accumulate 0 into e_ps so HAM is warm for real work
            for w in range(WARM):
                for m in range(2):
                    nc.tensor.matmul(
                        e_ps[m][:],
                        zet[:, m * 128 : (m + 1) * 128],
                        zmov[:],
                        start=(w == 0),
                        stop=False,
                    )

            sc = cst[:, SC_OFF : SC_OFF + NLOC * J2]

            # ---- phase 1: scale + accumulate over all 79 v-tiles ----
            # DVE scales 3 of every 5 v-tiles, ACT the other 2 (both notes
            # on one engine -> single-producer mov tiles).  The last TAIL
            # v-tiles emit all m=0 matmuls before the m=1 ones so the m=0
            # bias/cast of phase 2 overlaps the final m=1 accumulation.
            TAIL = 3
            tail_mm = []
            jg = 0
            for b, nb in enumerate(BLOCKS):
                for jj in range(nb):
                    j = jg + jj
                    wts = ww_b[b][:, jj * CK : jj * CK + C]
                    mov = movp.tile([128, NC2], BF16)
                    if j % 5 < 3:
                        nc.vector.tensor_scalar_mul(
                            mov[:, 0:C], wts, sc[:, j : j + 1]
                        )
                        nc.vector.tensor_scalar_mul(
                            mov[:, C : 2 * C], wts, sc[:, J2 + j : J2 + j + 1]
                        )
                    else:
                        nc.scalar.mul(mov[:, 0:C], wts, mul=sc[:, j : j + 1])
                        nc.scalar.mul(
                            mov[:, C : 2 * C], wts, mul=sc[:, J2 + j : J2 + j + 1]
                        )
                    sp = j == J - 1
                    lhs = lambda m, _b=b, _jj=jj: ww_b[_b][
                        :,
                        _jj * CK + C + m * 128 : _jj * CK + C + (m + 1) * 128,
                    ]
                    nc.tensor.matmul(
                        e_ps[0][:], lhs(0), mov[:], start=False, stop=sp
                    )
                    if j >= J - TAIL:
                        tail_mm.append((lhs(1), mov, sp))
                    else:
                        nc.tensor.matmul(
                            e_ps[1][:], lhs(1), mov[:], start=False, stop=False
                        )
                jg += nb
            for lhs1, mov, sp in tail_mm:
                nc.tensor.matmul(e_ps[1][:], lhs1, mov[:], start=False, stop=sp)

            # ---- phase 2: bias, attn logits, sigmoid, gate, W_out ----
            be = cst[:, BE_OFF : BE_OFF + 2]
            ba = cst[:, BA_OFF : BA_OFF + 2]
            bo = cst[:, BO_OFF : BO_OFF + 1]

            zdum = psp.tile([128, NC2], F32, name="zdum", tag="zdum")

            NFILL = 4
            fill_i = [0]

            def filler(n, rhs):
                # matmuls into a scratch bank that keep the PE clock warm
                # through phase-2 dependency stalls.  One accumulation chain
                # (not dead-code-eliminated: zdum is read at the end) whose
                # rhs is the latest-produced tile, so the scheduler cannot
                # float them ahead of the stall they are meant to fill.
                for _ in range(n):
                    nc.tensor.matmul(
                        zdum[:],
                        zet[:, 0:128],
                        rhs,
                        start=(fill_i[0] == 0),
                        stop=(fill_i[0] == NFILL - 1),
                    )
                    fill_i[0] += 1

            # DVE/ACT cast the e^T halves to bf16 straight from PSUM
            # (b_emb's effect on the attn logits is folded into the b_att
            # column on the host, so no bias pass is needed here)
            eb0 = postp.tile([128, NC2], BF16, tag="eb0")
            nc.vector.tensor_copy(eb0[:], e_ps[0][:])
            eb1 = postp.tile([128, NC2], BF16, tag="eb1")
            nc.scalar.copy(eb1[:], e_ps[1][:])
            eb = [eb0, eb1]

            a_ps = [
                psp.tile([128, NC2], F32, name=f"a_ps{jm}", tag=f"a_ps{jm}")
                for jm in range(2)
            ]
            for jm in range(2):
                for kt in range(2):
                    nc.tensor.matmul(
                        a_ps[jm][:],
                        wat[:, kt * K + jm * 128 : kt * K + (jm + 1) * 128],
                        eb[kt][:],
                        start=(kt == 0),
                        stop=(kt == 1),
                    )

            # gate input e + b_emb, cast bf16 (off the critical path)
            ef = []
            for m in range(2):
                ef_m = postp.tile([128, NC2], BF16, tag=f"ef{m}")
                nc.vector.tensor_scalar_add(ef_m[:], e_ps[m][:], be[:, m : m + 1])
                ef.append(ef_m)

            s_ps = psp.tile([1, NC2], F32, tag="s_ps")
            for jm in range(2):
                atn = postp.tile([128, NC2], BF16, tag=f"atn{jm}")
                nc.scalar.activation(
                    atn[:],
                    a_ps[jm][:],
                    mybir.ActivationFunctionType.Sigmoid,
                    bias=ba[:, jm : jm + 1],
                    scale=1.0,
                )
                v_jm = postp.tile([128, NC2], BF16, tag=f"v{jm}")
                nc.vector.tensor_mul(v_jm[:], atn[:], ef[jm][:])
                if jm == 0:
                    # PE stays warm while ACT runs sigmoid 0 + DVE gates
                    filler(3, eb[0][:])
                nc.tensor.matmul(
                    s_ps[:],
                    wout[:, jm : jm + 1],
                    v_jm[:],
                    start=(jm == 0),
                    stop=(jm == 1),
                )
                if jm == 0:
                    filler(1, atn[:])
            s_sb = postp.tile([1, NC2], F32, tag="s_sb")
            nc.scalar.add(s_sb[:], s_ps[:], bo[0:1, 0:1])
            nc.sync.dma_start(s_out[:], s_sb[:])
            # liveness anchor for the filler matmuls
            zrd = postp.tile([1, 1], F32, tag="zrd")
            nc.vector.tensor_copy(zrd[:], zdum[0:1, 0:1])

    nc.compile()
    return nc


def _get_nc():
    if "nc" not in _NC_CACHE:
        _NC_CACHE["nc"] = _build_nc()
    return _NC_CACHE["nc"]


def _pack_ww(wiki, wemb):
    """-> [128, J*CK] bf16; per v-tile j: C wiki cols then K wemb cols,
    col value at partition p is a[c, j*128+p] (zero-padded past V)."""
    out = np.zeros((J * 128, CK), np.float32)
    out[:V, :C] = np.asarray(wiki, np.float32).T
    out[:V, C:] = np.asarray(wemb, np.float32).T
    out = out.reshape(J, 128, CK).transpose(1, 0, 2)
    return np.ascontiguousarray(out.reshape(128, J * CK)).astype(BF16_NP)


def prep_inputs(notevec, wikivec, W_emb, b_emb, W_att, b_att, W_out, b_out):
    wwT = _pack_ww(wikivec, W_emb)
    # watT[p, kt*K + jm*128 + q] must hold W_att[jm*128+q, kt*128+p]
    wa = np.asarray(W_att, np.float32)  # (j, k)
    watT = np.zeros((128, 2 * K), np.float32)
    for kt in range(2):
        for jm in range(2):
            watT[:, kt * K + jm * 128 : kt * K + (jm + 1) * 128] = wa[
                jm * 128 : (jm + 1) * 128, kt * 128 : (kt + 1) * 128
            ].T
    watT = watT.astype(BF16_NP)
    woutB = (
        np.ascontiguousarray(np.asarray(W_out, np.float32)[0].reshape(2, 128).T)
        .astype(BF16_NP)
    )

    nv = np.zeros((N, J2 * 128), np.float32)
    nv[:, :V] = np.asarray(notevec, np.float32)
    bemb = np.asarray(b_emb, np.float32).reshape(2, 128).T
    # b_emb's contribution to the attn logits, folded into b_att
    # (uses the bf16-rounded W_att actually applied on device)
    ba2 = np.asarray(b_att, np.float32) + wa.astype(BF16_NP).astype(
        np.float32
    ) @ np.asarray(b_emb, np.float32)
    batt = ba2.reshape(2, 128).T
    bo = float(np.asarray(b_out, np.float32).reshape(1)[0])

    in_maps = []
    for i in range(N_CORES):
        cst = np.zeros((128, CST_COLS), np.float32)
        # scales[p, l*J2 + j] = notevec[NLOC*i+l, j*128+p]
        scl = nv[i * NLOC : (i + 1) * NLOC].reshape(NLOC, J2, 128).transpose(2, 0, 1)
        cst[:, SC_OFF : SC_OFF + NLOC * J2] = scl.reshape(128, NLOC * J2)
        cst[:, BE_OFF : BE_OFF + 2] = bemb
        cst[:, BA_OFF : BA_OFF + 2] = batt
        cst[:, BO_OFF] = bo
        in_maps.append(
            {
                "wwT": wwT,
                "cstF": np.ascontiguousarray(cst),
                "watT": watT,
                "woutB": woutB,
            }
        )
    return in_maps


def run(in_maps, **kw):
    nc = _get_nc()
    return run_bass_kernel_spmd(nc, in_maps, list(range(N_CORES)), **kw)


def kernel(notevec, wikivec, W_emb, b_emb, W_att, b_att, W_out, b_out):
    in_maps = prep_inputs(
        notevec, wikivec, W_emb, b_emb, W_att, b_att, W_out, b_out
    )
    res = run(in_maps)
    out = np.concatenate(
        [r["s_out"].reshape(NLOC, C) for r in res.results], axis=0
    )
    return out.astype(np.float32)
